# revision 4
# baseline (speedup 1.0000x reference)
"""Trainium2 Bass kernel for nn_CLIP_multiloss (smooth-L1 CLIP loss).

Computes: vector = [labels | ones]; cos1 = vector@vector.T; cos2 = mod@mod.T;
ml = floor(cos1/cos2); loss = (smoothl1(o0, ml) + smoothl1(o1, ml)) / 2.

Sharding: row-parallel over the 4096 batch rows across 8 NeuronCores
(512 rows each). Each core computes its row-block of the similarity and
partial smooth-L1 sums; host sums the per-core partials.

Device algorithm (per core, rows R = 512, cols 4096):
  h[i,j]  = (labels[i]·labels[j] + 128) * inv_i - mod_j        (PE, 3 matmuls)
  s       = Sign(h)   in {-1,0,1};  ml = (s+1)/2 = floor(cos1/cos2)
  e'      = o - 0.5*s = e + 0.5  where e = o - ml              (DVE stt)
  smoothl1(e) = 0.5*min(e^2,1) + relu(e-1) + relu(-e-1)
    A = sum min(Square(e'-0.5),1)      (ACT Square + DVE min-accum)
    B = sum max(e',1.5)  -> sum relu(e-1)  = B - 1.5N          (DVE max-accum)
    C = sum min(e',-0.5) -> sum relu(-e-1) = -C - 0.5N         (DVE min-accum)
  loss = sum_cores (0.5A + B - C - 2N) / (2*4096^2)
"""
import numpy as np

B = 4096
L = 128
NCORES = 8
R = B // NCORES          # 512 rows per core
P = 128                  # partitions
RT = R // P              # 4 row-tiles per core
CHUNK = 2048             # column chunk (4 PSUM banks)
NCC = B // CHUNK         # 2 col chunks
NITER = RT * NCC         # 8 main iterations
N_E = 2 * R * B          # e-elements per core (both matrices)

_compiled = None


def _build():
    import concourse.bacc as bacc
    import concourse.tile as tile
    import concourse.mybir as mybir
    from concourse import masks

    F32 = mybir.dt.float32
    BF16 = mybir.dt.bfloat16
    ALU = mybir.AluOpType
    ACT = mybir.ActivationFunctionType

    nc = bacc.Bacc("TRN2", target_bir_lowering=False, debug=False)
    o0r = nc.dram_tensor("o0r", [R, B], F32, kind="ExternalInput")
    o1r = nc.dram_tensor("o1r", [R, B], F32, kind="ExternalInput")
    labr = nc.dram_tensor("labr", [R, L], F32, kind="ExternalInput")
    labels = nc.dram_tensor("labels", [B, L], F32, kind="ExternalInput")
    acc_out = nc.dram_tensor("acc_out", [P, 3 * NITER], F32, kind="ExternalOutput")

    with tile.TileContext(nc) as tc:
        with (
            tc.tile_pool(name="persist", bufs=1) as persist,
            tc.tile_pool(name="consts", bufs=1) as consts,
        ):
            ident = consts.tile([P, P], F32)
            masks.make_identity(nc, ident[:])
            ones_col_bf = consts.tile([P, 1], BF16)
            nc.vector.memset(ones_col_bf, 1.0)
            ones_row_bf = consts.tile([1, 512], BF16)
            nc.vector.memset(ones_row_bf, 1.0)
            neg_ones_row_bf = consts.tile([1, P], BF16)
            nc.vector.memset(neg_ones_row_bf, -1.0)
            b128 = consts.tile([1, 1], F32)
            nc.vector.memset(b128, 128.0)
            b128p = consts.tile([P, 1], F32)
            nc.vector.memset(b128p, 128.0)
            nm05 = consts.tile([P, 1], F32)
            nc.vector.memset(nm05, -0.5)

            labelsT_bf = persist.tile([P, B], BF16)       # [k, j] all columns
            labT_own_bf = persist.tile([P, R], BF16)      # [k, i] own rows
            # lhsT pieces per row-tile
            lhsT_scaled = persist.tile([P, RT, P], BF16)  # labelsT_own * inv_i
            lhsT2 = persist.tile([1, RT, P], BF16)        # 128 * inv_i
            mod_row_bf = persist.tile([1, B], BF16)       # mod_j row (rhs3)

            # ---------- setup: transpose labels (full) and own rows ----------
            with (
                tc.tile_pool(name="stage", bufs=3) as stage,
                tc.tile_pool(name="tp_ps", bufs=4, space="PSUM") as tp_ps,
            ):
                for t in range(B // P):
                    lt = stage.tile([P, L], F32)
                    nc.sync.dma_start(lt, labels[P * t:P * (t + 1), :])
                    pst = tp_ps.tile([P, P], F32)
                    nc.tensor.transpose(pst[:], lt[:], ident[:])
                    nc.vector.tensor_copy(labelsT_bf[:, P * t:P * (t + 1)], pst)
                for t in range(RT):
                    lt = stage.tile([P, L], F32)
                    nc.sync.dma_start(lt, labr[P * t:P * (t + 1), :])
                    pst = tp_ps.tile([P, P], F32)
                    nc.tensor.transpose(pst[:], lt[:], ident[:])
                    nc.vector.tensor_copy(labT_own_bf[:, P * t:P * (t + 1)], pst)

            # ---------- setup: mod_row over all columns ----------
            labelsT_sq = persist.tile([P, B], BF16)
            nc.vector.tensor_tensor(labelsT_sq, labelsT_bf, labelsT_bf, ALU.mult)
            with tc.tile_pool(name="ssq_ps", bufs=1, space="PSUM") as ssq_ps:
                ssq_row_ps = ssq_ps.tile([1, B], F32)
                for cch in range(B // 512):
                    nc.tensor.matmul(
                        ssq_row_ps[:, 512 * cch:512 * (cch + 1)],
                        ones_col_bf, labelsT_sq[:, 512 * cch:512 * (cch + 1)],
                        start=True, stop=True)
                mod_row = persist.tile([1, B], F32)
                nc.scalar.activation(mod_row, ssq_row_ps, ACT.Sqrt, bias=b128)
            nc.vector.tensor_copy(mod_row_bf, mod_row)

            # ---------- setup: per-row-tile inv pieces ----------
            labT_own_sq = persist.tile([P, R], BF16)
            nc.vector.tensor_tensor(labT_own_sq, labT_own_bf, labT_own_bf, ALU.mult)
            with tc.tile_pool(name="inv_ps", bufs=2, space="PSUM") as inv_ps:
                mod_own_col = persist.tile([P, RT], F32)
                for rt in range(RT):
                    ssq_c = inv_ps.tile([P, 1], F32)
                    nc.tensor.matmul(ssq_c, labT_own_sq[:, P * rt:P * (rt + 1)],
                                     ones_col_bf, start=True, stop=True)
                    nc.scalar.activation(mod_own_col[:, rt:rt + 1], ssq_c,
                                         ACT.Sqrt, bias=b128p)
                inv_own_col = persist.tile([P, RT], F32)
                nc.vector.reciprocal(inv_own_col, mod_own_col)
                ir_bf = persist.tile([1, RT, P], BF16)
                for rt in range(RT):
                    # inv_row [1, P] via PE transpose of inv_own_col[:, rt]
                    ir_ps = inv_ps.tile([1, P], F32)
                    nc.tensor.transpose(ir_ps[:], inv_own_col[:, rt:rt + 1],
                                        ident[:])
                    # lhsT2 = 128 * inv_row (bf16)
                    nc.vector.tensor_scalar(lhsT2[:, rt, :], ir_ps, 128.0, None,
                                            ALU.mult)
                    # inv_bcast [P(k), P(i)] = ones_col x inv_row
                    nc.vector.tensor_copy(ir_bf[:, rt, :], ir_ps)
                    ib_ps = inv_ps.tile([P, P], F32)
                    nc.tensor.matmul(ib_ps, ones_row_bf[:, 0:P],
                                     ir_bf[:, rt, :], start=True, stop=True)
                    nc.vector.tensor_tensor(lhsT_scaled[:, rt, :],
                                            labT_own_bf[:, P * rt:P * (rt + 1)],
                                            ib_ps, ALU.mult)

            # ---------- main loop ----------
            acc = persist.tile([P, 3 * NITER], F32)
            with (
                tc.tile_pool(name="h_ps", bufs=2, space="PSUM") as h_ps,
                tc.tile_pool(name="sbuf", bufs=3) as sbuf,
                tc.tile_pool(name="scrap", bufs=2) as scrap,
            ):
                it = 0
                for rt in range(RT):
                    for cc in range(NCC):
                        c0 = CHUNK * cc
                        h = h_ps.tile([P, CHUNK], F32)
                        for sc in range(CHUNK // 512):
                            sl = slice(512 * sc, 512 * (sc + 1))
                            gsl = slice(c0 + 512 * sc, c0 + 512 * (sc + 1))
                            nc.tensor.matmul(h[:, sl], lhsT_scaled[:, rt, :],
                                             labelsT_bf[:, gsl],
                                             start=True, stop=False)
                            nc.tensor.matmul(h[:, sl], lhsT2[:, rt, :],
                                             ones_row_bf,
                                             start=False, stop=False)
                            nc.tensor.matmul(h[:, sl], neg_ones_row_bf,
                                             mod_row_bf[:, gsl],
                                             start=False, stop=True)
                        s_bf = sbuf.tile([P, CHUNK], BF16, tag="s")
                        nc.scalar.activation(s_bf, h, ACT.Sign)

                        obf = sbuf.tile([P, 2 * CHUNK], BF16, tag="o")
                        nc.gpsimd.dma_start(
                            obf[:, 0:CHUNK],
                            o0r[P * rt:P * (rt + 1), c0:c0 + CHUNK])
                        nc.gpsimd.dma_start(
                            obf[:, CHUNK:2 * CHUNK],
                            o1r[P * rt:P * (rt + 1), c0:c0 + CHUNK])

                        ep = sbuf.tile([P, 2 * CHUNK], BF16, tag="ep")
                        nc.vector.scalar_tensor_tensor(
                            ep[:, 0:CHUNK], s_bf, -0.5, obf[:, 0:CHUNK],
                            op0=ALU.mult, op1=ALU.add)
                        nc.vector.scalar_tensor_tensor(
                            ep[:, CHUNK:2 * CHUNK], s_bf, -0.5,
                            obf[:, CHUNK:2 * CHUNK],
                            op0=ALU.mult, op1=ALU.add)

                        q = sbuf.tile([P, 2 * CHUNK], BF16, tag="q")
                        nc.scalar.activation(q, ep, ACT.Square, bias=nm05)

                        sc1 = scrap.tile([P, 2 * CHUNK], BF16, tag="sc1")
                        nc.vector.tensor_scalar(sc1, q, 1.0, None,
                                                ALU.min, ALU.add,
                                                accum_out=acc[:, 3 * it:3 * it + 1])
                        sc2 = scrap.tile([P, 2 * CHUNK], BF16, tag="sc2")
                        nc.vector.tensor_scalar(sc2, ep, 1.5, None,
                                                ALU.max, ALU.add,
                                                accum_out=acc[:, 3 * it + 1:3 * it + 2])
                        sc3 = scrap.tile([P, 2 * CHUNK], BF16, tag="sc3")
                        nc.vector.tensor_scalar(sc3, ep, -0.5, None,
                                                ALU.min, ALU.add,
                                                accum_out=acc[:, 3 * it + 2:3 * it + 3])
                        it += 1

            nc.sync.dma_start(acc_out[:, :], acc)
    nc.finalize()
    return nc


def kernel(outputs0, outputs1, labels):
    global _compiled
    from concourse.bass_utils import run_bass_kernel_spmd

    outputs0 = np.ascontiguousarray(np.asarray(outputs0, dtype=np.float32))
    outputs1 = np.ascontiguousarray(np.asarray(outputs1, dtype=np.float32))
    labels = np.ascontiguousarray(np.asarray(labels, dtype=np.float32))

    if _compiled is None:
        _compiled = _build()
    nc = _compiled

    in_maps = []
    for c in range(NCORES):
        rows = slice(c * R, (c + 1) * R)
        in_maps.append({
            "o0r": np.ascontiguousarray(outputs0[rows]),
            "o1r": np.ascontiguousarray(outputs1[rows]),
            "labr": np.ascontiguousarray(labels[rows]),
            "labels": labels,
        })
    res = run_bass_kernel_spmd(nc, in_maps, core_ids=list(range(NCORES)))

    total = 0.0
    for c in range(NCORES):
        acc = res.results[c]["acc_out"].astype(np.float64)
        acc = acc.reshape(P, NITER, 3)
        A = acc[:, :, 0].sum()
        Bm = acc[:, :, 1].sum()
        C = acc[:, :, 2].sum()
        total += 0.5 * A + Bm - C - 2.0 * N_E
    loss = total / (2.0 * B * B)
    return np.float32(loss)


# revision 5
# speedup vs baseline: 1.9847x; 1.9847x over previous
"""Trainium2 Bass kernel for nn_CLIP_multiloss (smooth-L1 CLIP loss).

Computes: vector = [labels | ones]; cos1 = vector@vector.T; cos2 = mod@mod.T;
ml = floor(cos1/cos2); loss = (smoothl1(o0, ml) + smoothl1(o1, ml)) / 2.

Sharding: row-parallel over the 4096 batch rows across 8 NeuronCores
(512 rows each). Each core computes its row-block of the similarity and
partial smooth-L1 sums; host sums the per-core partials.

Device algorithm (per core, rows R = 512, cols 4096):
  h[i,j]  = (labels[i]·labels[j] + 128) * inv_i - mod_j        (PE, 3 matmuls)
  s       = Sign(h)   in {-1,0,1};  ml = (s+1)/2 = floor(cos1/cos2)
  e'      = o - 0.5*s = e + 0.5  where e = o - ml              (DVE stt)
  smoothl1(e) = 0.5*min(e^2,1) + relu(e-1) + relu(-e-1)
    A = sum min(Square(e'-0.5),1)      (ACT Square + DVE min-accum)
    B = sum max(e',1.5)  -> sum relu(e-1)  = B - 1.5N          (DVE max-accum)
    C = sum min(e',-0.5) -> sum relu(-e-1) = -C - 0.5N         (DVE min-accum)
  loss = sum_cores (0.5A + B - C - 2N) / (2*4096^2)
"""
import numpy as np

B = 4096
L = 128
NCORES = 8
R = B // NCORES          # 512 rows per core
P = 128                  # partitions
RT = R // P              # 4 row-tiles per core
CHUNK = 2048             # column chunk (4 PSUM banks)
NCC = B // CHUNK         # 2 col chunks
NITER = RT * NCC         # 8 main iterations
N_E = 2 * R * B          # e-elements per core (both matrices)

_compiled = None


def _build(loop_reps=1):
    import concourse.bacc as bacc
    import concourse.tile as tile
    import concourse.mybir as mybir
    from concourse import masks

    F32 = mybir.dt.float32
    BF16 = mybir.dt.bfloat16
    ALU = mybir.AluOpType
    ACT = mybir.ActivationFunctionType

    nc = bacc.Bacc("TRN2", target_bir_lowering=False, debug=False)
    o0r = nc.dram_tensor("o0r", [R, B], F32, kind="ExternalInput")
    o1r = nc.dram_tensor("o1r", [R, B], F32, kind="ExternalInput")
    labr = nc.dram_tensor("labr", [R, L], F32, kind="ExternalInput")
    labels = nc.dram_tensor("labels", [B, L], F32, kind="ExternalInput")
    acc_out = nc.dram_tensor("acc_out", [P, 3 * NITER], F32, kind="ExternalOutput")

    with tile.TileContext(nc) as tc:
        with (
            tc.tile_pool(name="persist", bufs=1) as persist,
            tc.tile_pool(name="consts", bufs=1) as consts,
        ):
            ident = consts.tile([P, P], F32)
            masks.make_identity(nc, ident[:])
            ones_col_bf = consts.tile([P, 1], BF16)
            nc.vector.memset(ones_col_bf, 1.0)
            ones_row_bf = consts.tile([1, 512], BF16)
            nc.vector.memset(ones_row_bf, 1.0)
            neg_ones_row_bf = consts.tile([1, P], BF16)
            nc.vector.memset(neg_ones_row_bf, -1.0)
            b128 = consts.tile([1, 1], F32)
            nc.vector.memset(b128, 128.0)
            b128p = consts.tile([P, 1], F32)
            nc.vector.memset(b128p, 128.0)
            nm05 = consts.tile([P, 1], F32)
            nc.vector.memset(nm05, -0.5)

            labelsT_bf = persist.tile([P, B], BF16)       # [k, j] all columns
            labT_own_bf = persist.tile([P, R], BF16)      # [k, i] own rows
            # lhsT pieces per row-tile
            lhsT_scaled = persist.tile([P, RT, P], BF16)  # labelsT_own * inv_i
            lhsT2 = persist.tile([1, RT, P], BF16)        # 128 * inv_i
            mod_row_bf = persist.tile([1, B], BF16)       # mod_j row (rhs3)

            # ---------- setup: transpose labels (full) and own rows ----------
            with (
                tc.tile_pool(name="stage", bufs=3) as stage,
                tc.tile_pool(name="tp_ps", bufs=4, space="PSUM") as tp_ps,
            ):
                for t in range(B // P):
                    lt = stage.tile([P, L], F32)
                    nc.sync.dma_start(lt, labels[P * t:P * (t + 1), :])
                    pst = tp_ps.tile([P, P], F32)
                    nc.tensor.transpose(pst[:], lt[:], ident[:])
                    nc.vector.tensor_copy(labelsT_bf[:, P * t:P * (t + 1)], pst)
                for t in range(RT):
                    lt = stage.tile([P, L], F32)
                    nc.sync.dma_start(lt, labr[P * t:P * (t + 1), :])
                    pst = tp_ps.tile([P, P], F32)
                    nc.tensor.transpose(pst[:], lt[:], ident[:])
                    nc.vector.tensor_copy(labT_own_bf[:, P * t:P * (t + 1)], pst)

            # ---------- setup: mod_row over all columns ----------
            labelsT_sq = persist.tile([P, B], BF16)
            nc.vector.tensor_tensor(labelsT_sq, labelsT_bf, labelsT_bf, ALU.mult)
            with tc.tile_pool(name="ssq_ps", bufs=1, space="PSUM") as ssq_ps:
                ssq_row_ps = ssq_ps.tile([1, B], F32)
                for cch in range(B // 512):
                    nc.tensor.matmul(
                        ssq_row_ps[:, 512 * cch:512 * (cch + 1)],
                        ones_col_bf, labelsT_sq[:, 512 * cch:512 * (cch + 1)],
                        start=True, stop=True)
                mod_row = persist.tile([1, B], F32)
                nc.scalar.activation(mod_row, ssq_row_ps, ACT.Sqrt, bias=b128)
            nc.vector.tensor_copy(mod_row_bf, mod_row)

            # ---------- setup: per-row-tile inv pieces ----------
            labT_own_sq = persist.tile([P, R], BF16)
            nc.vector.tensor_tensor(labT_own_sq, labT_own_bf, labT_own_bf, ALU.mult)
            with tc.tile_pool(name="inv_ps", bufs=2, space="PSUM") as inv_ps:
                mod_own_col = persist.tile([P, RT], F32)
                for rt in range(RT):
                    ssq_c = inv_ps.tile([P, 1], F32)
                    nc.tensor.matmul(ssq_c, labT_own_sq[:, P * rt:P * (rt + 1)],
                                     ones_col_bf, start=True, stop=True)
                    nc.scalar.activation(mod_own_col[:, rt:rt + 1], ssq_c,
                                         ACT.Sqrt, bias=b128p)
                inv_own_col = persist.tile([P, RT], F32)
                nc.vector.reciprocal(inv_own_col, mod_own_col)
                ir_bf = persist.tile([1, RT, P], BF16)
                for rt in range(RT):
                    # inv_row [1, P] via PE transpose of inv_own_col[:, rt]
                    ir_ps = inv_ps.tile([1, P], F32)
                    nc.tensor.transpose(ir_ps[:], inv_own_col[:, rt:rt + 1],
                                        ident[:])
                    # lhsT2 = 128 * inv_row (bf16)
                    nc.vector.tensor_scalar(lhsT2[:, rt, :], ir_ps, 128.0, None,
                                            ALU.mult)
                    # inv_bcast [P(k), P(i)] = ones_col x inv_row
                    nc.vector.tensor_copy(ir_bf[:, rt, :], ir_ps)
                    ib_ps = inv_ps.tile([P, P], F32)
                    nc.tensor.matmul(ib_ps, ones_row_bf[:, 0:P],
                                     ir_bf[:, rt, :], start=True, stop=True)
                    nc.vector.tensor_tensor(lhsT_scaled[:, rt, :],
                                            labT_own_bf[:, P * rt:P * (rt + 1)],
                                            ib_ps, ALU.mult)

            # ---------- main loop ----------
            acc = persist.tile([P, 3 * NITER], F32)
            with (
                tc.tile_pool(name="h_ps", bufs=2, space="PSUM") as h_ps,
                tc.tile_pool(name="sbuf", bufs=3) as sbuf,
                tc.tile_pool(name="scrap", bufs=2) as scrap,
            ):
              for _rep in range(loop_reps):
                it = 0
                for rt in range(RT):
                    for cc in range(NCC):
                        c0 = CHUNK * cc
                        h = h_ps.tile([P, CHUNK], F32)
                        for sc in range(CHUNK // 512):
                            sl = slice(512 * sc, 512 * (sc + 1))
                            gsl = slice(c0 + 512 * sc, c0 + 512 * (sc + 1))
                            nc.tensor.matmul(h[:, sl], lhsT_scaled[:, rt, :],
                                             labelsT_bf[:, gsl],
                                             start=True, stop=False)
                            nc.tensor.matmul(h[:, sl], lhsT2[:, rt, :],
                                             ones_row_bf,
                                             start=False, stop=False)
                            nc.tensor.matmul(h[:, sl], neg_ones_row_bf,
                                             mod_row_bf[:, gsl],
                                             start=False, stop=True)
                        s_bf = sbuf.tile([P, CHUNK], BF16, tag="s")
                        nc.scalar.activation(s_bf, h, ACT.Sign)

                        obf = sbuf.tile([P, 2 * CHUNK], BF16, tag="o")
                        nc.gpsimd.dma_start(
                            obf[:, 0:CHUNK],
                            o0r[P * rt:P * (rt + 1), c0:c0 + CHUNK])
                        nc.gpsimd.dma_start(
                            obf[:, CHUNK:2 * CHUNK],
                            o1r[P * rt:P * (rt + 1), c0:c0 + CHUNK])

                        ep = sbuf.tile([P, 2 * CHUNK], BF16, tag="ep")
                        nc.vector.scalar_tensor_tensor(
                            ep[:, 0:CHUNK], s_bf, -0.5, obf[:, 0:CHUNK],
                            op0=ALU.mult, op1=ALU.add)
                        nc.vector.scalar_tensor_tensor(
                            ep[:, CHUNK:2 * CHUNK], s_bf, -0.5,
                            obf[:, CHUNK:2 * CHUNK],
                            op0=ALU.mult, op1=ALU.add)

                        q = sbuf.tile([P, 2 * CHUNK], BF16, tag="q")
                        nc.scalar.activation(q, ep, ACT.Square, bias=nm05)

                        sc1 = scrap.tile([P, 2 * CHUNK], BF16, tag="sc1")
                        nc.vector.tensor_scalar(sc1, q, 1.0, None,
                                                ALU.min, ALU.add,
                                                accum_out=acc[:, 3 * it:3 * it + 1])
                        sc2 = scrap.tile([P, 2 * CHUNK], BF16, tag="sc2")
                        nc.vector.tensor_scalar(sc2, ep, 1.5, None,
                                                ALU.max, ALU.add,
                                                accum_out=acc[:, 3 * it + 1:3 * it + 2])
                        sc3 = scrap.tile([P, 2 * CHUNK], BF16, tag="sc3")
                        nc.vector.tensor_scalar(sc3, ep, -0.5, None,
                                                ALU.min, ALU.add,
                                                accum_out=acc[:, 3 * it + 2:3 * it + 3])
                        it += 1

            nc.sync.dma_start(acc_out[:, :], acc)
    nc.finalize()
    return nc


def kernel(outputs0, outputs1, labels):
    global _compiled
    from concourse.bass_utils import run_bass_kernel_spmd

    outputs0 = np.ascontiguousarray(np.asarray(outputs0, dtype=np.float32))
    outputs1 = np.ascontiguousarray(np.asarray(outputs1, dtype=np.float32))
    labels = np.ascontiguousarray(np.asarray(labels, dtype=np.float32))

    if _compiled is None:
        _compiled = _build()
    nc = _compiled

    in_maps = []
    for c in range(NCORES):
        rows = slice(c * R, (c + 1) * R)
        in_maps.append({
            "o0r": np.ascontiguousarray(outputs0[rows]),
            "o1r": np.ascontiguousarray(outputs1[rows]),
            "labr": np.ascontiguousarray(labels[rows]),
            "labels": labels,
        })
    res = run_bass_kernel_spmd(nc, in_maps, core_ids=list(range(NCORES)))

    total = 0.0
    for c in range(NCORES):
        acc = res.results[c]["acc_out"].astype(np.float64)
        acc = acc.reshape(P, NITER, 3)
        A = acc[:, :, 0].sum()
        Bm = acc[:, :, 1].sum()
        C = acc[:, :, 2].sum()
        total += 0.5 * A + Bm - C - 2.0 * N_E
    loss = total / (2.0 * B * B)
    return np.float32(loss)
